# revision 2
# baseline (speedup 1.0000x reference)
# DeepSet Trainium2 kernel, v2.
#
# Events sorted by jet-count n (2..10), round-robin sharded across 8 cores
# into per-group slots of capacity cap_g (multiple of 8). Within a group all
# masks/pair structures/counts are compile-time constants.
#
# Dense+BN+relu folded host-side into relu(h @ W' + b'); MLP2 layer 1 via
# z-trick: y1 = relu(z_i + z_j + t), z = x @ Wz'.
#
# v2 structure: 2-stage software pipeline over groups — stage s emits
#   [DVE y1(s-1)] [jets mms+evacs (s)] [pairs mms/evacs/aggs (s-1)]
#   [transposes+out (s-2)]
# so PE never waits on DVE y1 within a stage. All activations bf16 (f32 in
# PSUM), pair adds use stride-0 broadcast APs (one DVE op per anchor jet),
# squares/x-max on GpSimd, sums via PSUM-accumulating identity matmuls.
import math
from contextlib import ExitStack

import numpy as np

import concourse.bass as bass
import concourse.bacc as bacc
import concourse.tile as tile
import concourse.mybir as mybir

f32 = mybir.dt.float32
bf16 = mybir.dt.bfloat16
f32r = mybir.dt.float32r
fp8 = mybir.dt.float8e4
AF = mybir.ActivationFunctionType
ALU = mybir.AluOpType

H = 128
FJ = 16

# ---- engine-assignment knobs ----
# NOTE: gpsimd supports only 1-input ops (tensor_scalar/copy) in this
# toolchain — tensor_tensor on Pool fails the backend engine check.
EVAC_Y3 = "alternate"  # "scalar" | "vector" | "alternate" (per window)
YSQ_ENG = "vector"
YMAX_ENG = "vector"
XSQ_ENG = "vector"
XMAX_ENG = "vector"
Y1_RELU_ENG = "vector"  # gpsimd TS measured ~6ns/col + SBUF-port poison: no
PAIR_MM_DT = bf16    # fp8 end-to-end measured rel_err 3.2e-2 (> 2e-2): no
MM_CHUNK = 1024      # PSUM evac tile width (2 banks); 1536×2bufs ping-pongs


def pairs_of(g):
    return [(i, j) for i in range(g) for j in range(i + 1, g)]


def build_program(groups, pair_mm_dt=PAIR_MM_DT):
    """groups: list of (g, cap); cap multiple of 8, cap <= 512."""
    act = bf16
    JC = sum(g * cap for g, cap in groups)
    EC = sum(cap for _, cap in groups)

    nc = bacc.Bacc("TRN2", target_bir_lowering=False, debug=False)

    jets_d = nc.dram_tensor("jets", [FJ, JC], act, kind="ExternalInput")
    w1_d = nc.dram_tensor("w1", [FJ, H], act, kind="ExternalInput")
    w2_d = nc.dram_tensor("w2", [H, H], act, kind="ExternalInput")
    w3_d = nc.dram_tensor("w3", [H, H], act, kind="ExternalInput")
    wz_d = nc.dram_tensor("wz", [H, H], act, kind="ExternalInput")
    w4_d = nc.dram_tensor("w4", [H, H], pair_mm_dt, kind="ExternalInput")
    w5_d = nc.dram_tensor("w5", [H, H], pair_mm_dt, kind="ExternalInput")
    identp_d = nc.dram_tensor("identp", [H, H], act, kind="ExternalInput")
    identt_d = nc.dram_tensor("identt", [H, H], f32, kind="ExternalInput")
    # bias cols: 0..5 = b1, b2, b3, t21 (y1), b4, b5
    bv_d = nc.dram_tensor("bvec", [H, 8], f32, kind="ExternalInput")
    outx_d = nc.dram_tensor("outx", [EC, 4 * H], f32, kind="ExternalOutput")
    outy_d = nc.dram_tensor("outy", [EC, 4 * H], f32, kind="ExternalOutput")

    n_g = len(groups)

    with tile.TileContext(nc) as tc, ExitStack() as ctx:
        consts = ctx.enter_context(tc.tile_pool(name="consts", bufs=1))
        jin = ctx.enter_context(tc.tile_pool(name="jin", bufs=2))
        x12p = ctx.enter_context(tc.tile_pool(name="x12", bufs=2))
        xzp = ctx.enter_context(tc.tile_pool(name="xz", bufs=2))
        y1p = ctx.enter_context(tc.tile_pool(name="y1", bufs=2))
        y2p = ctx.enter_context(tc.tile_pool(name="y2", bufs=3))
        y3p = ctx.enter_context(tc.tile_pool(name="y3", bufs=1))
        sqp = ctx.enter_context(tc.tile_pool(name="sq", bufs=2))
        mxp = ctx.enter_context(tc.tile_pool(name="mxp", bufs=2))
        aggp = ctx.enter_context(tc.tile_pool(name="agg", bufs=2))
        outp = ctx.enter_context(tc.tile_pool(name="outp", bufs=2))
        mm = ctx.enter_context(tc.tile_pool(name="mm", bufs=3, space="PSUM"))
        accp = ctx.enter_context(tc.tile_pool(name="acc", bufs=1, space="PSUM"))

        def const_tile(name, dram, shape, dt):
            t = consts.tile(shape, dt, tag=name)
            nc.sync.dma_start(t[:], dram.ap())
            return t

        w1t = const_tile("w1", w1_d, [FJ, H], act)
        w2t = const_tile("w2", w2_d, [H, H], act)
        w3t = const_tile("w3", w3_d, [H, H], act)
        wzt = const_tile("wz", wz_d, [H, H], act)
        w4t = const_tile("w4", w4_d, [H, H], pair_mm_dt)
        w5t = const_tile("w5", w5_d, [H, H], pair_mm_dt)
        ip_t = const_tile("ip", identp_d, [H, H], act)
        it_t = const_tile("it", identt_d, [H, H], f32)
        bv = const_tile("bv", bv_d, [H, 8], f32)

        jets_offs = []
        ev_offs = []
        jo = eo = 0
        for g, cap in groups:
            jets_offs.append(jo)
            ev_offs.append(eo)
            jo += g * cap
            eo += cap

        # per-stage state passed between pipeline phases
        st = [dict() for _ in range(n_g)]

        def rr(ap, k):
            return ap.rearrange("p (k c) -> p k c", k=k)

        # ---------------- jets phase: L1..L3, z matmuls + evacs ----------
        def _layer(gi, dst, wt, src, bias_col, evac, relu=True):
            g, cap = groups[gi]
            JCg = g * cap
            tiles = []
            for c0 in range(0, JCg, MM_CHUNK):
                w = min(MM_CHUNK, JCg - c0)
                ps = mm.tile([H, MM_CHUNK], f32, tag="mm")
                for s0 in range(0, w, 512):
                    sw = min(512, w - s0)
                    nc.tensor.matmul(ps[:, s0 : s0 + sw], wt[:],
                                     src[:, c0 + s0 : c0 + s0 + sw],
                                     start=True, stop=True)
                tiles.append((ps, c0, w))
            for ps, c0, w in tiles:
                if evac == "scalar":
                    if relu:
                        nc.scalar.activation(dst[:, c0 : c0 + w], ps[:, :w],
                                             AF.Relu,
                                             bias=bv[:, bias_col : bias_col + 1])
                    else:
                        nc.scalar.copy(dst[:, c0 : c0 + w], ps[:, :w])
                elif relu:
                    nc.vector.tensor_scalar(
                        dst[:, c0 : c0 + w], ps[:, :w],
                        bv[:, bias_col : bias_col + 1], 0.0, ALU.add, ALU.max)
                else:
                    nc.vector.tensor_scalar(
                        dst[:, c0 : c0 + w], ps[:, :w],
                        bv[:, bias_col : bias_col + 1], None, ALU.add)

        def emit_jets(gi):
            g, cap = groups[gi]
            JCg = g * cap
            jt = jin.tile([FJ, JCg], act, tag="jt")
            nc.sync.dma_start(jt[:], jets_d.ap()[:, jets_offs[gi] : jets_offs[gi] + JCg])
            x1 = x12p.tile([H, JCg], act, tag="x1")
            x2 = x12p.tile([H, JCg], act, tag="x2")
            x = xzp.tile([H, JCg], act, tag="x")
            z = xzp.tile([H, JCg], act, tag="z")
            _layer(gi, x1, w1t, jt, 0, "scalar")
            _layer(gi, x2, w2t, x1, 1, "scalar")
            _layer(gi, x, w3t, x2, 2, "scalar")
            # z = Wz.T x (plain copy evac; t21 bias applied in the y1 relu TS)
            _layer(gi, z, wzt, x, 3, "scalar", relu=False)
            st[gi]["x"] = x
            st[gi]["z"] = z

        # ---------------- y1 phase (DVE) ---------------------------------
        def emit_y1(gi):
            g, cap = groups[gi]
            z = st[gi]["z"]
            prs = pairs_of(g)
            PG = len(prs)
            y1 = y1p.tile([H, PG * cap], PAIR_MM_DT, tag="y1")
            # pairs are lexicographic: for anchor i, js are contiguous
            off = 0
            for i in range(g - 1):
                k = g - 1 - i
                zi = z[:, i * cap : (i + 1) * cap].rearrange(
                    "p (k c) -> p k c", k=1).broadcast_to([H, k, cap])
                nc.vector.tensor_tensor(
                    rr(y1[:, off * cap : (off + k) * cap], k), zi,
                    rr(z[:, (i + 1) * cap : g * cap], k), ALU.add)
                off += k
            # bias t21 + relu in one TS pass (add then max)
            W = PG * cap
            reng = nc.gpsimd if Y1_RELU_ENG == "gpsimd" else nc.vector
            reng.tensor_scalar(y1[:, 0:W], y1[:, 0:W],
                               bv[:, 3:4], 0.0, ALU.add, ALU.max)
            st[gi]["y1"] = y1

        # ---------------- pairs phase ------------------------------------
        def emit_pairs(gi):
            g, cap = groups[gi]
            prs = pairs_of(g)
            PG = len(prs)
            W = PG * cap
            y1 = st[gi]["y1"]
            x = st[gi]["x"]

            # max trees — flat 2D contiguous APs so bf16 gets 2x mode
            def max_tree(src, nsl, dt, eng, tag, out_slot):
                m = nsl
                cur, cur_off = src, 0
                if m == 1:
                    nc.vector.tensor_copy(out_slot, src[:, 0:cap])
                    return
                while m > 1:
                    k2 = (m + 1) // 2
                    a0 = cur[:, cur_off : cur_off + k2 * cap]
                    a1 = cur[:, cur_off + (m - k2) * cap : cur_off + m * cap]
                    if k2 == 1:
                        nc.vector.tensor_tensor(out_slot, a0, a1, ALU.max)
                        return
                    nxt = mxp.tile([H, k2 * cap], dt, tag=tag)
                    eng.tensor_tensor(nxt[:, 0 : k2 * cap], a0, a1, ALU.max)
                    cur, cur_off, m = nxt, 0, k2

            # aggregate slots: 8 quantities x cap, bf16 (all blocks tolerate
            # 0.4% of their own scale vs the global-scale error metric;
            # sums/means/vars are still ACCUMULATED in f32)
            agg = aggp.tile([H, 8 * cap], act, tag="agg")
            SX, MXX, MNX, VRX = 0, 1, 2, 3
            SY, MXY, MNY, VRY = 4, 5, 6, 7

            def slot(q):
                return agg[:, q * cap : (q + 1) * cap]

            # DVE-early work depending only on x(gi) (ready last stage):
            JCg = g * cap
            xsq = sqp.tile([H, JCg], act, tag="xsq")
            for c0 in range(0, JCg, 4096):
                w = min(4096, JCg - c0)
                nc.vector.tensor_mul(xsq[:, c0 : c0 + w], x[:, c0 : c0 + w],
                                     x[:, c0 : c0 + w])
            max_tree(x, g, act, nc.vector, "mxx", slot(MXX))

            y3 = y3p.tile([H, W], act, tag="y3")

            # y2/y3 in 1024-col windows, software-pipelined one window deep:
            # PE runs y2mm(w+1) while Scalar evacs y2(w); y3mm(w) then has
            # its input ready without stalling PE.
            def emit_y2(c0):
                w = min(MM_CHUNK, W - c0)
                ps = mm.tile([H, MM_CHUNK], f32, tag="mm")
                for s0 in range(0, w, 512):
                    sw = min(512, w - s0)
                    nc.tensor.matmul(ps[:, s0 : s0 + sw], w4t[:],
                                     y1[:, c0 + s0 : c0 + s0 + sw],
                                     start=True, stop=True)
                y2 = y2p.tile([H, MM_CHUNK], pair_mm_dt, tag="y2")
                nc.scalar.activation(y2[:, :w], ps[:, :w], AF.Relu,
                                     bias=bv[:, 4:5])
                return (y2, c0, w)

            def emit_y3(t):
                y2, c0, w = t
                ps = mm.tile([H, MM_CHUNK], f32, tag="mm")
                for s0 in range(0, w, 512):
                    sw = min(512, w - s0)
                    nc.tensor.matmul(ps[:, s0 : s0 + sw], w5t[:],
                                     y2[:, s0 : s0 + sw], start=True, stop=True)
                eng = EVAC_Y3
                if eng == "alternate":
                    eng = "vector" if (c0 // MM_CHUNK) % 3 == 0 else "scalar"
                if eng == "scalar":
                    nc.scalar.activation(y3[:, c0 : c0 + w], ps[:, :w], AF.Relu,
                                         bias=bv[:, 5:6])
                else:
                    nc.vector.tensor_scalar(y3[:, c0 : c0 + w], ps[:, :w],
                                            bv[:, 5:6], 0.0, ALU.add, ALU.max)

            from collections import deque
            pend = deque()
            for c0 in range(0, W, MM_CHUNK):
                pend.append(emit_y2(c0))
                if len(pend) > 2:
                    emit_y3(pend.popleft())
            while pend:
                emit_y3(pend.popleft())

            # PSUM accumulators: [x | xsq] halves and [y | ysq] halves
            accx = accp.tile([H, 2 * cap], f32, tag="accx")
            accy = accp.tile([H, 2 * cap], f32, tag="accy")

            # Σy identity-mms (slice-per-instr, accumulate into accy[0:cap])
            for s in range(PG):
                nc.tensor.matmul(accy[:, 0:cap], ip_t[:],
                                 y3[:, s * cap : (s + 1) * cap],
                                 start=(s == 0), stop=(s == PG - 1))
            # ysq slabs + Σy² mms
            eng = nc.gpsimd if YSQ_ENG == "gpsimd" else nc.vector
            SLAB = 8  # slices per ysq slab
            for s0 in range(0, PG, SLAB):
                k = min(SLAB, PG - s0)
                ysq = sqp.tile([H, SLAB * cap], act, tag="ysq")
                eng.tensor_mul(ysq[:, : k * cap],
                               y3[:, s0 * cap : (s0 + k) * cap],
                               y3[:, s0 * cap : (s0 + k) * cap])
                for s in range(k):
                    nc.tensor.matmul(accy[:, cap : 2 * cap], ip_t[:],
                                     ysq[:, s * cap : (s + 1) * cap],
                                     start=(s0 + s == 0),
                                     stop=(s0 + s == PG - 1))

            # Σx / Σx² identity-mms (late: xsq long since done)
            for s in range(g):
                nc.tensor.matmul(accx[:, 0:cap], ip_t[:],
                                 x[:, s * cap : (s + 1) * cap],
                                 start=(s == 0), stop=(s == g - 1))
            for s in range(g):
                nc.tensor.matmul(accx[:, cap : 2 * cap], ip_t[:],
                                 xsq[:, s * cap : (s + 1) * cap],
                                 start=(s == 0), stop=(s == g - 1))

            max_tree(y3, PG, act, nc.vector, "mxy", slot(MXY))

            # sums + means/vars from PSUM accumulators (DVE; one PSUM operand)
            inv_g = 1.0 / g
            inv_p = 1.0 / PG
            nc.vector.tensor_copy(slot(SX), accx[:, 0:cap])
            nc.vector.tensor_scalar(slot(MNX), accx[:, 0:cap], inv_g, None,
                                    ALU.mult)
            nc.vector.tensor_scalar(slot(VRX), accx[:, cap : 2 * cap], inv_g,
                                    None, ALU.mult)  # q/g (var pre)
            nc.vector.tensor_copy(slot(SY), accy[:, 0:cap])
            nc.vector.tensor_scalar(slot(MNY), accy[:, 0:cap], inv_p, None,
                                    ALU.mult)
            nc.vector.tensor_scalar(slot(VRY), accy[:, cap : 2 * cap], inv_p,
                                    None, ALU.mult)
            # var = q/n - mean^2 : msq then subtract (scratch in mxp pool)
            msq = mxp.tile([H, 2 * cap], act, tag="msq")
            nc.vector.tensor_mul(msq[:, 0:cap], slot(MNX), slot(MNX))
            nc.vector.tensor_mul(msq[:, cap : 2 * cap], slot(MNY), slot(MNY))
            nc.vector.tensor_sub(slot(VRX), slot(VRX), msq[:, 0:cap])
            nc.vector.tensor_sub(slot(VRY), slot(VRY), msq[:, cap : 2 * cap])
            st[gi]["agg"] = agg

        # ---------------- output phase: transpose + stage + DMA ----------
        def emit_out(gi):
            g, cap = groups[gi]
            agg = st[gi]["agg"]
            for t0 in range(0, cap, 128):
                tw = min(128, cap - t0)
                tp = mm.tile([tw, 1024], act, tag="mm")
                for q in range(8):
                    nc.tensor.transpose(
                        tp[:, q * 128 : q * 128 + 128],
                        agg[:, q * cap + t0 : q * cap + t0 + tw], ip_t[:])
                ot = outp.tile([tw, 1024], f32, tag="ot")
                nc.scalar.copy(ot[:], tp[:])
                r0 = ev_offs[gi] + t0
                nc.sync.dma_start(outx_d.ap()[r0 : r0 + tw, :], ot[:, 0:512])
                nc.sync.dma_start(outy_d.ap()[r0 : r0 + tw, :], ot[:, 512:1024])
            st[gi].clear()

        # ---------------- pipelined emission -----------------------------
        # Stage s: jets(s) -> y1(s) [DVE, overlaps Scalar's pair evacs of
        # s-1] -> pairs(s-1) -> out(s-2). y2 matmuls of stage s+1 find
        # y1(s) ready; DVE's y1 block overlaps Scalar's evac block.
        for s in range(n_g + 2):
            if s < n_g:
                emit_jets(s)
                emit_y1(s)
            if 1 <= s <= n_g:
                emit_pairs(s - 1)
            if 2 <= s:
                emit_out(s - 2)

    nc.compile()
    return nc


# ---------------- host-side math ----------------

BN_EPS = 1e-3


def fold_params(inp):
    mean_j = np.asarray(inp["mean_jets"], np.float32)
    std_j = np.asarray(inp["std_jets"], np.float32)
    w1f = np.asarray(inp["w1_first"], np.float32)
    w1r = np.asarray(inp["w1_rest"], np.float32)
    bn1 = np.asarray(inp["bn1"], np.float32)
    w2f = np.asarray(inp["w2_first"], np.float32)
    w2r = np.asarray(inp["w2_rest"], np.float32)
    bn2 = np.asarray(inp["bn2"], np.float32)

    def bn_sb(row):
        gm, bt, mu, vv = row[0], row[1], row[2], row[3]
        s = gm / np.sqrt(vv + BN_EPS)
        return s.astype(np.float32), (bt - mu * s).astype(np.float32)

    s11, t11 = bn_sb(bn1[0]); s12, t12 = bn_sb(bn1[1]); s13, t13 = bn_sb(bn1[2])
    s21, t21 = bn_sb(bn2[0]); s22, t22 = bn_sb(bn2[1]); s23, t23 = bn_sb(bn2[2])

    A = w1f / std_j[:, None]
    c = -(mean_j / std_j) @ w1f
    return dict(
        W1=A * s11[None, :], b1=c * s11 + t11,
        W2=w1r[0] * s12[None, :], b2=t12,
        W3=w1r[1] * s13[None, :], b3=t13,
        Wz=w2f * s21[None, :], bz=t21,
        W4=w2r[0] * s22[None, :], b4=t22,
        W5=w2r[1] * s23[None, :], b5=t23,
    )


# ---------------- full kernel entry point ----------------

N_CORES = 8

_cache = {}
_TRACE = [False]
_LAST_RESULT = [None]


def _get_program(groups_key):
    key = (groups_key, PAIR_MM_DT)
    if key not in _cache:
        _cache[key] = build_program(list(groups_key))
    return _cache[key]


def _np_dt(dt):
    return mybir.dt.np(dt)


def _plan(n):
    gs = []
    idx_by_g = {}
    for g in range(2, 11):
        idx = np.nonzero(n == g)[0]
        if len(idx):
            gs.append(g)
            idx_by_g[g] = idx
    stray = np.nonzero((n < 2) | (n > 10))[0]
    if len(stray):
        if not gs:
            gs.append(2)
            idx_by_g[2] = stray
        else:
            idx_by_g[gs[-1]] = np.concatenate([idx_by_g[gs[-1]], stray])
    gs.sort(key=lambda g: -(g * (g - 1) // 2))  # big groups first
    groups = []
    slots = [[] for _ in range(N_CORES)]
    for g in gs:
        idx = idx_by_g[g]
        per_core = [idx[c::N_CORES] for c in range(N_CORES)]
        mx = max(len(p) for p in per_core)
        cap = max(32, ((mx + 7) // 8) * 8)
        groups.append((g, cap))
        fill = idx[0]
        for c in range(N_CORES):
            p = per_core[c]
            pad = np.full(cap, p[0] if len(p) else fill, dtype=np.int64)
            pad[: len(p)] = p
            slots[c].append((pad, len(p)))
    return groups, slots


def _pack_jets(jets, groups, slots_c, np_dt):
    cols = []
    for (g, cap), (ids, _cnt) in zip(groups, slots_c):
        ev = jets[ids][:, :g, :]  # [cap, g, 16]
        cols.append(np.ascontiguousarray(ev.transpose(2, 1, 0)).reshape(
            FJ, g * cap))
    return np.concatenate(cols, axis=1).astype(np_dt, copy=False)


def kernel(**inputs):
    from concourse.bass_utils import run_bass_kernel_spmd

    jets = np.asarray(inputs["inputs_jets"], dtype=np.float32)
    B = jets.shape[0]
    mask = (jets != 0.0).any(-1)
    n = mask.sum(-1).astype(np.int64)
    if not np.array_equal(mask, np.arange(jets.shape[1])[None, :] < n[:, None]):
        order = np.argsort(~mask, axis=1, kind="stable")
        jets = np.take_along_axis(jets, order[:, :, None], axis=1)

    P = fold_params(inputs)
    groups, slots = _plan(n)
    nc = _get_program(tuple(groups))

    bvec = np.zeros((H, 8), np.float32)
    for i, k in enumerate(["b1", "b2", "b3"]):
        bvec[:, i] = P[k]
    bvec[:, 3] = P["bz"]  # t21, applied in the y1 relu TS pass
    bvec[:, 4] = P["b4"]
    bvec[:, 5] = P["b5"]
    ident = np.eye(H, dtype=np.float32)
    a_np = _np_dt(bf16)
    p_np = _np_dt(PAIR_MM_DT)
    common = {
        "w1": P["W1"].astype(a_np), "w2": P["W2"].astype(a_np),
        "w3": P["W3"].astype(a_np), "wz": P["Wz"].astype(a_np),
        "w4": P["W4"].astype(p_np), "w5": P["W5"].astype(p_np),
        "identp": ident.astype(a_np), "identt": ident, "bvec": bvec,
    }
    in_maps = []
    for c in range(N_CORES):
        m = dict(common)
        m["jets"] = _pack_jets(jets, groups, slots[c], a_np)
        in_maps.append(m)

    res = run_bass_kernel_spmd(nc, in_maps, core_ids=list(range(N_CORES)),
                               trace=_TRACE[0])
    _LAST_RESULT[0] = res

    agg_x = np.empty((B, 4 * H), np.float32)
    agg_y = np.empty((B, 4 * H), np.float32)
    for c in range(N_CORES):
        ox = res.results[c]["outx"]
        oy = res.results[c]["outy"]
        ev_off = 0
        for (g, cap), (ids, cnt) in zip(groups, slots[c]):
            agg_x[ids[:cnt]] = ox[ev_off : ev_off + cnt]
            agg_y[ids[:cnt]] = oy[ev_off : ev_off + cnt]
            ev_off += cap
    return agg_x, agg_y
